# revision 11
# baseline (speedup 1.0000x reference)
"""DeepPoly conv transformer kernel for 8 TRN2 NeuronCores.

Computes, for conv(C_in=8,H=W=32 -> C_out=32,K=4,S=2,P=1):
  - out_bounds [2,32,16,16]: interval bounds through the conv (DeepPoly forward
    + terminal back-substitution; both are mathematically identical here)
  - W_mat [8192,8192]: the Toeplitz (jacobian) matrix of the conv
  - bias_backsub [8192]: bias repeated per output pixel

Strategy: rows of W_mat (out_feat dim) are sharded across the 8 cores (4
c_out channels each). W_mat is ~98% zeros; row (co,oy,ox) holds weight
values at ci*1024 + (2oy-1+ky)*32 + (2ox-1+kx). Each core:
  1. zero-fills its 33.5MB row-block with big contiguous DMAs,
  2. builds, in SBUF, per (ci,oy)-partition "band images": 128-element runs
     covering the 4 iy-rows a row touches in one ci slab, with the weight
     values pre-placed at their ox-shifted positions (placement uses a
     free-dim stride of 130 = 128+2, so the conv's diagonal shift is encoded
     at SBUF-build time by 16 tiny strided DVE copies),
  3. scatters those 512B runs into DRAM with 3-dim strided APs (row stride
     8192, oy stride 131136 = 16*8192+64) - 40 DMAs, ~8K descriptors total.
Boundary rows oy=0/15 use clipped 96-element runs; kx=0/3 edge columns are
clipped in the SBUF build, so no write ever leaves its own row (no
cross-row write-write races). The small conv bounds are computed on the
TensorEngine via accumulating K=8 matmuls from a zero-padded input staged
in SBUF.
"""
import numpy as np

import bass_rust
import concourse.bass as bass
import concourse.tile as tile
from concourse import mybir, bacc
from concourse.bass_utils import run_bass_kernel_spmd

# problem constants (hardcoded per spec)
C_IN, H, W = 8, 32, 32
C_OUT, K = 32, 4
HO = WO = 16
IN_FEAT = C_IN * H * W            # 8192
N_CORES = 8
CO_PER_CORE = C_OUT // N_CORES    # 4
ROWS_PER_BLOCK = HO * WO          # 256 rows per c_out block
GUARD = 8192
BLOCK_ELEMS = ROWS_PER_BLOCK * IN_FEAT      # 2097152
WT_FLAT = GUARD + BLOCK_ELEMS + GUARD       # per-block output tensor size

OY_STEP = HO * IN_FEAT + 2 * W    # 131136: next oy row block (+64 col shift)

VP = bass_rust.VecI64Pair


def _cap(ap, dims, offset):
    a = ap.copy()
    a.ap = VP(dims)
    a.offset = offset
    return a


def build_bass():
    nc = bacc.Bacc("TRN2", target_bir_lowering=False, debug=False,
                   num_devices=N_CORES)
    f32 = mybir.dt.float32
    w_in = nc.declare_dram_parameter("w", [CO_PER_CORE * C_IN * K * K], f32, isOutput=False)
    b_in = nc.declare_dram_parameter("b", [CO_PER_CORE], f32, isOutput=False)
    bounds_in = nc.declare_dram_parameter("bounds", [2 * IN_FEAT], f32, isOutput=False)
    wt = [nc.declare_dram_parameter(f"w{co}", [WT_FLAT], f32, isOutput=True)
          for co in range(CO_PER_CORE)]
    ob_out = nc.declare_dram_parameter("ob", [2 * CO_PER_CORE * ROWS_PER_BLOCK], f32, isOutput=True)
    bb_out = nc.declare_dram_parameter("bb", [CO_PER_CORE * ROWS_PER_BLOCK], f32, isOutput=True)

    with tile.TileContext(nc) as tc:
        with tc.tile_pool(name="big", bufs=1) as big, \
             tc.tile_pool(name="small", bufs=1) as small, \
             tc.tile_pool(name="ps", bufs=1, space="PSUM") as ps:

            # ---- zero source tile ----
            ztile = big.tile([128, 4096], f32)          # 2MB zero source
            nc.gpsimd.memset(ztile[:], 0.0)

            # w5[p=(ci,oy), (co,ky,kx)] = w[co,ci,ky,kx]  (broadcast over oy)
            w5 = small.tile([128, 64], f32)
            for ci in range(C_IN):
                dst = _cap(w5[:], [[64, 16], [1, 64]], ci * 16 * 64)
                src = _cap(w_in[:], [[0, 16], [128, CO_PER_CORE], [1, 16]], ci * 16)
                nc.sync.dma_start(dst, src)

            # ---- band-image tile ----
            # m2[p=(ci,oy), co*2048 + ox*128 + j] = value of W row (co,oy,ox)
            # at ci-slab offset (2oy-1)*32 + j   (j in [0,128))
            m2 = big.tile([128, 8192], f32)
            nc.vector.memset(m2[:], 0.0)
            for ky in range(K):
                for kx in range(K):
                    if kx == 0:
                        ox0, n_ox = 1, 15     # ox=0,kx=0 is out-of-image
                    elif kx == 3:
                        ox0, n_ox = 0, 15     # ox=15,kx=3 is out-of-image
                    else:
                        ox0, n_ox = 0, 16
                    # value (ky,kx) sits at j = ky*32 + 2*ox-1+kx
                    out = _cap(m2[:], [[8192, 128], [2048, CO_PER_CORE], [130, n_ox], [1, 1]],
                               ox0 * 130 + ky * 32 + kx - 1)
                    src = _cap(w5[:], [[64, 128], [16, CO_PER_CORE], [0, n_ox], [1, 1]],
                               ky * K + kx)
                    nc.vector.tensor_copy(out, src)

            # ---- conv bounds inputs ----
            bp = small.tile([C_IN, 2 * 34 * 34], f32)   # zero-padded l|u
            nc.vector.memset(bp[:], 0.0)
            for lu in range(2):
                dst = _cap(bp[:], [[2312, C_IN], [34, H], [1, W]], lu * 1156 + 35)
                src = _cap(bounds_in[:], [[1024, C_IN], [32, H], [1, W]], lu * IN_FEAT)
                nc.sync.dma_start(dst, src)
            w2 = small.tile([C_IN, 64], f32)            # [ci, (co,ky,kx)]
            for co in range(CO_PER_CORE):
                dst = _cap(w2[:], [[64, C_IN], [1, 16]], co * 16)
                src = _cap(w_in[:], [[16, C_IN], [1, 16]], co * 128)
                nc.sync.dma_start(dst, src)
            w2p = small.tile([C_IN, 64], f32)
            nc.vector.tensor_scalar_max(w2p[:], w2[:], 0.0)
            w2m = small.tile([C_IN, 64], f32)
            nc.vector.tensor_tensor(w2m[:], w2[:], w2p[:], op=mybir.AluOpType.subtract)
            bias_t = small.tile([CO_PER_CORE, 1], f32)
            nc.sync.dma_start(bias_t[:], b_in[:].rearrange("(a b) -> a b", b=1))

            # ---- bulk zero-fill: 4 chunks of [128,4096] per c_out block ----
            with tc.high_priority():
                for co in range(CO_PER_CORE):
                    for chunk in range(4):
                        dst = _cap(wt[co][:], [[4096, 128], [1, 4096]],
                                   GUARD + chunk * 128 * 4096)
                        nc.scalar.dma_start(dst, ztile[:])

            # ---- scatter: 512B band runs on sync, split by oy halves so each
            # DMA depends only on 2 of the 4 zero chunks of its block ----
            for co in range(CO_PER_CORE):
                # block 3's scatters split across sync+scalar (they issue last)
                eng2 = nc.scalar if co == CO_PER_CORE - 1 else nc.sync
                # edge rows oy=0: 96-elem runs, all ci in one DMA (chunk 0)
                src = _cap(m2[:], [[131072, C_IN], [128, WO], [1, 96]],
                           co * 2048 + 32)
                dst = _cap(wt[co][:], [[1024, C_IN], [8192, WO], [1, 96]], GUARD)
                nc.sync.dma_start(dst, src)
                # interior rows, first halves oy=1..7 (chunks 0-1), then 8..14
                for oy0, n_oy in ((1, 7), (8, 7)):
                    for ci in range(C_IN):
                        src = _cap(m2[:], [[8192, n_oy], [128, WO], [1, 128]],
                                   (ci * 16 + oy0) * 8192 + co * 2048)
                        dst = _cap(wt[co][:], [[OY_STEP, n_oy], [8192, WO], [1, 128]],
                                   GUARD + oy0 * 16 * 8192 + ci * 1024 + (2 * oy0 - 1) * 32)
                        eng = eng2 if ci % 2 else nc.sync
                        eng.dma_start(dst, src)
                # edge rows oy=15: 96-elem runs (chunk 3)
                src = _cap(m2[:], [[131072, C_IN], [128, WO], [1, 96]],
                           15 * 8192 + co * 2048)
                dst = _cap(wt[co][:], [[1024, C_IN], [8192, WO], [1, 96]],
                           GUARD + 240 * 8192 + 29 * 32)
                eng2.dma_start(dst, src)

            # ---- conv bounds via accumulating matmuls ----
            # ps_lu cols [0:256] = Wp@l + Wm@u ("low"), [256:512] = Wp@u + Wm@l
            ps_lu = ps.tile([CO_PER_CORE, 512], f32)
            out_mm = _cap(ps_lu[:], [[512, CO_PER_CORE], [256, 2], [16, HO], [1, WO]], 0)
            idx = 0
            for ky in range(K):
                for kx in range(K):
                    lp = _cap(w2p[:], [[64, C_IN], [16, CO_PER_CORE]], ky * K + kx)
                    lm = _cap(w2m[:], [[64, C_IN], [16, CO_PER_CORE]], ky * K + kx)
                    # rhs [l|u] and swapped [u|l] via +/-1156-stride lu dim
                    xlu = _cap(bp[:], [[2312, C_IN], [1156, 2], [68, HO], [2, WO]],
                               ky * 34 + kx)
                    xul = _cap(bp[:], [[2312, C_IN], [-1156, 2], [68, HO], [2, WO]],
                               1156 + ky * 34 + kx)
                    first, last = idx == 0, idx == 15
                    nc.tensor.matmul(out_mm, lp, xlu, start=first, stop=False)
                    nc.tensor.matmul(out_mm, lm, xul, start=False, stop=last)
                    idx += 1

            ob_sb = small.tile([CO_PER_CORE, 512], f32)
            nc.vector.tensor_scalar(ob_sb[:], ps_lu[:], bias_t[:], None,
                                    op0=mybir.AluOpType.add)
            dst = _cap(ob_out[:], [[256, CO_PER_CORE], [1024, 2], [1, 256]], 0)
            src = _cap(ob_sb[:], [[512, CO_PER_CORE], [256, 2], [1, 256]], 0)
            nc.scalar.dma_start(dst, src)

            bb_sb = small.tile([CO_PER_CORE, 256], f32)
            nc.vector.tensor_scalar(bb_sb[:], ztile[0:CO_PER_CORE, 0:256], bias_t[:],
                                    None, op0=mybir.AluOpType.add)
            nc.scalar.dma_start(bb_out[:].rearrange("(a b) -> a b", a=CO_PER_CORE), bb_sb[:])

    nc.compile()
    return nc


_NC = None


def kernel(bounds, weight, bias, assignment):
    global _NC
    if _NC is None:
        _NC = build_bass()
    bounds = np.ascontiguousarray(bounds, np.float32)
    weight = np.ascontiguousarray(weight, np.float32)
    bias = np.ascontiguousarray(bias, np.float32)

    in_maps = []
    for c in range(N_CORES):
        sl = slice(c * CO_PER_CORE, (c + 1) * CO_PER_CORE)
        in_maps.append({
            "w": weight[sl].reshape(-1),
            "b": bias[sl].copy(),
            "bounds": bounds.reshape(-1),
        })
    res = run_bass_kernel_spmd(_NC, in_maps, list(range(N_CORES)))

    W_mat = np.empty((C_OUT * ROWS_PER_BLOCK, IN_FEAT), np.float32)
    out_bounds = np.empty((2, C_OUT, HO, WO), np.float32)
    bias_backsub = np.empty(C_OUT * ROWS_PER_BLOCK, np.float32)
    for c in range(N_CORES):
        r = res.results[c]
        for co in range(CO_PER_CORE):
            base = (c * CO_PER_CORE + co) * ROWS_PER_BLOCK
            W_mat[base:base + ROWS_PER_BLOCK] = (
                r[f"w{co}"][GUARD:GUARD + BLOCK_ELEMS].reshape(ROWS_PER_BLOCK, IN_FEAT))
        ob = r["ob"].reshape(2, CO_PER_CORE, HO, WO)
        out_bounds[:, c * CO_PER_CORE:(c + 1) * CO_PER_CORE] = ob
        bias_backsub[c * CO_PER_CORE * 256:(c + 1) * CO_PER_CORE * 256] = r["bb"]
    return out_bounds, W_mat, bias_backsub


# revision 15
# speedup vs baseline: 1.2141x; 1.2141x over previous
"""DeepPoly conv transformer kernel for 8 TRN2 NeuronCores.

Computes, for conv(C_in=8,H=W=32 -> C_out=32,K=4,S=2,P=1):
  - out_bounds [2,32,16,16]: interval bounds through the conv (DeepPoly forward
    + terminal back-substitution; both are mathematically identical here)
  - W_mat [8192,8192]: the Toeplitz (jacobian) matrix of the conv
  - bias_backsub [8192]: bias repeated per output pixel

Strategy: rows of W_mat (out_feat dim) are sharded across the 8 cores (4
c_out channels each). W_mat is ~98% zeros; row (co,oy,ox) holds weight
values at ci*1024 + (2oy-1+ky)*32 + (2ox-1+kx). Each core:
  1. zero-fills its 33.5MB row-block with big contiguous DMAs,
  2. builds, in SBUF, per (ci,oy)-partition "band images": 128-element runs
     covering the 4 iy-rows a row touches in one ci slab, with the weight
     values pre-placed at their ox-shifted positions (placement uses a
     free-dim stride of 130 = 128+2, so the conv's diagonal shift is encoded
     at SBUF-build time by 16 tiny strided DVE copies),
  3. scatters those 512B runs into DRAM with 3-dim strided APs (row stride
     8192, oy stride 131136 = 16*8192+64) - 40 DMAs, ~8K descriptors total.
Boundary rows oy=0/15 use clipped 96-element runs; kx=0/3 edge columns are
clipped in the SBUF build, so no write ever leaves its own row (no
cross-row write-write races). The small conv bounds are computed on the
TensorEngine via accumulating K=8 matmuls from a zero-padded input staged
in SBUF.
"""
import numpy as np

import bass_rust
import concourse.bass as bass
import concourse.tile as tile
from concourse import mybir, bacc
from concourse.bass_utils import run_bass_kernel_spmd

# problem constants (hardcoded per spec)
C_IN, H, W = 8, 32, 32
C_OUT, K = 32, 4
HO = WO = 16
IN_FEAT = C_IN * H * W            # 8192
N_CORES = 8
CO_PER_CORE = C_OUT // N_CORES    # 4
ROWS_PER_BLOCK = HO * WO          # 256 rows per c_out block
GUARD = 8192
BLOCK_ELEMS = ROWS_PER_BLOCK * IN_FEAT      # 2097152
WT_FLAT = GUARD + BLOCK_ELEMS + GUARD       # per-block output tensor size

OY_STEP = HO * IN_FEAT + 2 * W    # 131136: next oy row block (+64 col shift)

VP = bass_rust.VecI64Pair


def _cap(ap, dims, offset):
    a = ap.copy()
    a.ap = VP(dims)
    a.offset = offset
    return a


def build_bass():
    nc = bacc.Bacc("TRN2", target_bir_lowering=False, debug=False,
                   num_devices=N_CORES)
    f32 = mybir.dt.float32
    w_in = nc.declare_dram_parameter("w", [CO_PER_CORE * C_IN * K * K], f32, isOutput=False)
    b_in = nc.declare_dram_parameter("b", [CO_PER_CORE], f32, isOutput=False)
    bounds_in = nc.declare_dram_parameter("bounds", [2 * IN_FEAT], f32, isOutput=False)
    wt = [nc.declare_dram_parameter(f"w{co}", [WT_FLAT], f32, isOutput=True)
          for co in range(CO_PER_CORE)]
    ob_out = nc.declare_dram_parameter("ob", [2 * CO_PER_CORE * ROWS_PER_BLOCK], f32, isOutput=True)
    bb_out = nc.declare_dram_parameter("bb", [CO_PER_CORE * ROWS_PER_BLOCK], f32, isOutput=True)

    with tile.TileContext(nc) as tc:
        with tc.tile_pool(name="big", bufs=1) as big, \
             tc.tile_pool(name="small", bufs=1) as small, \
             tc.tile_pool(name="ps", bufs=1, space="PSUM") as ps:

            # ---- zero source tile ----
            ztile = big.tile([128, 8192], f32)          # 4MB zero source
            nc.gpsimd.memset(ztile[:], 0.0)

            # w5[p=(ci,oy), (co,ky,kx)] = w[co,ci,ky,kx]  (broadcast over oy)
            w5 = small.tile([128, 64], f32)
            for ci in range(C_IN):
                dst = _cap(w5[:], [[64, 16], [1, 64]], ci * 16 * 64)
                src = _cap(w_in[:], [[0, 16], [128, CO_PER_CORE], [1, 16]], ci * 16)
                nc.gpsimd.dma_start(dst, src)

            # ---- band-image tile ----
            # m2[p=(ci,oy), co*2048 + ox*128 + j] = value of W row (co,oy,ox)
            # at ci-slab offset (2oy-1)*32 + j   (j in [0,128))
            m2 = big.tile([128, 8192], f32)
            nc.vector.memset(m2[:], 0.0)
            for ky in range(K):
                for kx in range(K):
                    if kx == 0:
                        ox0, n_ox = 1, 15     # ox=0,kx=0 is out-of-image
                    elif kx == 3:
                        ox0, n_ox = 0, 15     # ox=15,kx=3 is out-of-image
                    else:
                        ox0, n_ox = 0, 16
                    # value (ky,kx) sits at j = ky*32 + 2*ox-1+kx
                    out = _cap(m2[:], [[8192, 128], [2048, CO_PER_CORE], [130, n_ox], [1, 1]],
                               ox0 * 130 + ky * 32 + kx - 1)
                    src = _cap(w5[:], [[64, 128], [16, CO_PER_CORE], [0, n_ox], [1, 1]],
                               ky * K + kx)
                    nc.vector.tensor_copy(out, src)

            # ---- conv bounds inputs ----
            bp = small.tile([C_IN, 2 * 34 * 34], f32)   # zero-padded l|u
            nc.vector.memset(bp[:], 0.0)
            for lu in range(2):
                dst = _cap(bp[:], [[2312, C_IN], [34, H], [1, W]], lu * 1156 + 35)
                src = _cap(bounds_in[:], [[1024, C_IN], [32, H], [1, W]], lu * IN_FEAT)
                nc.sync.dma_start(dst, src)
            w2 = small.tile([C_IN, 64], f32)            # [ci, (co,ky,kx)]
            nc.sync.dma_start(
                _cap(w2[:], [[64, C_IN], [16, CO_PER_CORE], [1, 16]], 0),
                _cap(w_in[:], [[16, C_IN], [128, CO_PER_CORE], [1, 16]], 0))
            w2p = small.tile([C_IN, 64], f32)
            nc.vector.tensor_scalar_max(w2p[:], w2[:], 0.0)
            w2m = small.tile([C_IN, 64], f32)
            nc.vector.tensor_tensor(w2m[:], w2[:], w2p[:], op=mybir.AluOpType.subtract)
            bias_t = small.tile([CO_PER_CORE, 1], f32)
            nc.sync.dma_start(bias_t[:], b_in[:].rearrange("(a b) -> a b", b=1))

            # ---- bulk zero-fill: 2 chunks of [128,8192] per c_out block ----
            with tc.high_priority():
                for co in range(CO_PER_CORE):
                    for chunk in range(2):
                        dst = _cap(wt[co][:], [[8192, 128], [1, 8192]],
                                   GUARD + chunk * 128 * 8192)
                        nc.scalar.dma_start(dst, ztile[:])

            # ---- scatter: 512B band runs on sync, split by oy halves so each
            # DMA depends only on 2 of the 4 zero chunks of its block ----
            for co in range(CO_PER_CORE):
                # edge rows oy=0: 96-elem runs, all ci in one DMA (chunk 0)
                src = _cap(m2[:], [[131072, C_IN], [128, WO], [1, 96]],
                           co * 2048 + 32)
                dst = _cap(wt[co][:], [[1024, C_IN], [8192, WO], [1, 96]], GUARD)
                nc.sync.dma_start(dst, src)
                # interior rows oy=1..14, alternating engines by ci
                for ci in range(C_IN):
                    src = _cap(m2[:], [[8192, 14], [128, WO], [1, 128]],
                               (ci * 16 + 1) * 8192 + co * 2048)
                    dst = _cap(wt[co][:], [[OY_STEP, 14], [8192, WO], [1, 128]],
                               GUARD + 16 * 8192 + ci * 1024 + 32)
                    eng = nc.scalar if ci % 2 else nc.sync
                    eng.dma_start(dst, src)
                # edge rows oy=15: 96-elem runs (chunk 1)
                src = _cap(m2[:], [[131072, C_IN], [128, WO], [1, 96]],
                           15 * 8192 + co * 2048)
                dst = _cap(wt[co][:], [[1024, C_IN], [8192, WO], [1, 96]],
                           GUARD + 240 * 8192 + 29 * 32)
                nc.scalar.dma_start(dst, src)

            # ---- conv bounds via accumulating matmuls ----
            # ps_lu cols [0:256] = Wp@l + Wm@u ("low"), [256:512] = Wp@u + Wm@l
            ps_lu = ps.tile([CO_PER_CORE, 512], f32)
            out_mm = _cap(ps_lu[:], [[512, CO_PER_CORE], [256, 2], [16, HO], [1, WO]], 0)
            idx = 0
            for ky in range(K):
                for kx in range(K):
                    lp = _cap(w2p[:], [[64, C_IN], [16, CO_PER_CORE]], ky * K + kx)
                    lm = _cap(w2m[:], [[64, C_IN], [16, CO_PER_CORE]], ky * K + kx)
                    # rhs [l|u] and swapped [u|l] via +/-1156-stride lu dim
                    xlu = _cap(bp[:], [[2312, C_IN], [1156, 2], [68, HO], [2, WO]],
                               ky * 34 + kx)
                    xul = _cap(bp[:], [[2312, C_IN], [-1156, 2], [68, HO], [2, WO]],
                               1156 + ky * 34 + kx)
                    first, last = idx == 0, idx == 15
                    nc.tensor.matmul(out_mm, lp, xlu, start=first, stop=False)
                    nc.tensor.matmul(out_mm, lm, xul, start=False, stop=last)
                    idx += 1

            ob_sb = small.tile([CO_PER_CORE, 512], f32)
            nc.vector.tensor_scalar(ob_sb[:], ps_lu[:], bias_t[:], None,
                                    op0=mybir.AluOpType.add)
            dst = _cap(ob_out[:], [[256, CO_PER_CORE], [1024, 2], [1, 256]], 0)
            src = _cap(ob_sb[:], [[512, CO_PER_CORE], [256, 2], [1, 256]], 0)
            nc.scalar.dma_start(dst, src)

            bb_sb = small.tile([CO_PER_CORE, 256], f32)
            nc.vector.tensor_scalar(bb_sb[:], ztile[0:CO_PER_CORE, 0:256], bias_t[:],
                                    None, op0=mybir.AluOpType.add)
            nc.scalar.dma_start(bb_out[:].rearrange("(a b) -> a b", a=CO_PER_CORE), bb_sb[:])

    nc.compile()
    return nc


_NC = None


def kernel(bounds, weight, bias, assignment):
    global _NC
    if _NC is None:
        _NC = build_bass()
    bounds = np.ascontiguousarray(bounds, np.float32)
    weight = np.ascontiguousarray(weight, np.float32)
    bias = np.ascontiguousarray(bias, np.float32)

    in_maps = []
    for c in range(N_CORES):
        sl = slice(c * CO_PER_CORE, (c + 1) * CO_PER_CORE)
        in_maps.append({
            "w": weight[sl].reshape(-1),
            "b": bias[sl].copy(),
            "bounds": bounds.reshape(-1),
        })
    res = run_bass_kernel_spmd(_NC, in_maps, list(range(N_CORES)))

    W_mat = np.empty((C_OUT * ROWS_PER_BLOCK, IN_FEAT), np.float32)
    out_bounds = np.empty((2, C_OUT, HO, WO), np.float32)
    bias_backsub = np.empty(C_OUT * ROWS_PER_BLOCK, np.float32)
    for c in range(N_CORES):
        r = res.results[c]
        for co in range(CO_PER_CORE):
            base = (c * CO_PER_CORE + co) * ROWS_PER_BLOCK
            W_mat[base:base + ROWS_PER_BLOCK] = (
                r[f"w{co}"][GUARD:GUARD + BLOCK_ELEMS].reshape(ROWS_PER_BLOCK, IN_FEAT))
        ob = r["ob"].reshape(2, CO_PER_CORE, HO, WO)
        out_bounds[:, c * CO_PER_CORE:(c + 1) * CO_PER_CORE] = ob
        bias_backsub[c * CO_PER_CORE * 256:(c + 1) * CO_PER_CORE * 256] = r["bb"]
    return out_bounds, W_mat, bias_backsub
